# revision 24
# baseline (speedup 1.0000x reference)
"""GCN layer (GCNConv + relu + dense + relu) on 8 Trainium2 NeuronCores.

Strategy (v3.2: SBUF table + gpsimd ap_gather + shared-shape reduce)
-------------------------------------------------------------------
Math: out = relu(relu(GCNConv(x)) @ W_dense + b_dense) with
GCNConv(x)[v] = dinv[v] * sum_{e: s->v} dinv[s] * (x W_gcn)[s] + b_gcn
(self-loops as ordinary edges; dinv = rsqrt(indegree incl. self).)

Launch A (per core): g = dinv[node] * (x @ W_gcn), f32 node-major.
Host reassembles the global feature table
tab[32*s + f, 6251*q + j] = g[25000*s + 6250*q + j, f]  ([128, 25004]
f32) — partitions carry (source-subset s, feature f); each of the 4
quarters ends with a zero element (index 6250) used for slot padding.

Launch B (per core): the table lives in SBUF.  Four independent
"streams" (gpsimd core pairs, partitions [32s, 32s+32)) gather their
source subset with ONE ap_gather per batch.  Work is phased by source
QUARTER, so each instruction's table slice is only 6251 elements (the
gpsimd cost scales with the table operand).  Destinations are sorted
by per-(stream, quarter) in-degree; all 4 streams share each batch's
(m dests x K slots) shape, so a single [128, m, K] tensor_reduce sums
4 streams' dest runs at once into a bf16 accumulator that is flushed
to HBM after each quarter.  No mask, no per-edge DMA, no scatter.

Host re-aligns the 16 (stream, quarter) rank orders to canonical dest
order (pure permutation).  Launch C: 4 accumulating PE matmuls with an
indicator lhsT collapse quarters and subsets, then * dinv[dest],
+b_gcn, relu, @W_dense, +b_dense, relu -> [32, 12500] f32 per core.
Host transposes.
"""

import sys

if "/opt/trn_rl_repo" not in sys.path:
    sys.path.insert(0, "/opt/trn_rl_repo")

from dataclasses import dataclass

import ml_dtypes
import numpy as np

import concourse.bacc as bacc
import concourse.mybir as mybir
from concourse import tile
from concourse.bass_utils import run_bass_kernel_spmd

BF16 = ml_dtypes.bfloat16


@dataclass(frozen=True)
class Cfg:
    n_cores: int = 8
    nloc: int = 12500
    in_dim: int = 128
    net_dim: int = 32
    n_sub: int = 4            # source subsets = streams (32 partitions each)
    n_q: int = 4              # source quarters within a subset
    ni: int = 8192            # gather idxs per instruction per core section
    echunk: int = 512         # launch-C column chunk (one PSUM bank)

    @property
    def n(self):
        return self.nloc * self.n_cores

    @property
    def sub(self):
        return self.n // self.n_sub          # 25000 nodes per subset

    @property
    def qsub(self):
        return self.sub // self.n_q          # 6250 nodes per quarter

    @property
    def qelems(self):
        return self.qsub + 1                 # +1 zero element per quarter

    @property
    def elems(self):
        return self.n_q * self.qelems        # 25004 table columns


FULL = Cfg()
assert FULL.n == 100000 and FULL.sub == 25000 and FULL.qsub == 6250


def _f32(x):
    return np.ascontiguousarray(x, dtype=np.float32)


def _bf16(x):
    return np.ascontiguousarray(x, dtype=BF16)


# ------------------------------------------------------------ schedule


def build_schedule(kreq, ni):
    """kreq: [n_q, nloc], per-quarter per-rank slot needs (descending,
    shared by all streams and cores).  Returns batches [(q, segs)] where
    segs = ((r0, m, K, off), ...): each segment is a run of m dests with
    exactly K slots each, at slot offset `off` within the batch."""
    n_q, nloc = kreq.shape
    batches = []
    for q in range(n_q):
        kr = np.maximum(kreq[q], 1).astype(np.int64)
        segs, used = [], 0
        r = 0
        while r < nloc:
            k = int(kr[r])
            assert k <= ni
            r_end = r + int(np.searchsorted(-kr[r:], -k, side="right"))
            while r < r_end:
                cap = (ni - used) // k
                if cap == 0:
                    batches.append((q, tuple(segs)))
                    segs, used = [], 0
                    cap = ni // k
                m = min(cap, r_end - r)
                segs.append((r, m, k, used))
                used += m * k
                r += m
        if segs:
            batches.append((q, tuple(segs)))
    return batches


# ---------------------------------------------------------------- launch A


def build_launch_a(cfg: Cfg):
    nc = bacc.Bacc(
        "TRN2", target_bir_lowering=False, debug=False, num_devices=cfg.n_cores
    )
    K, F = cfg.in_dim, cfg.net_dim
    npad = -(-cfg.nloc // 1024) * 1024  # 12800... -> 13312? keep 512-mult
    npad = -(-cfg.nloc // 512) * 512    # 12800
    T = npad // 512
    xt_d = nc.dram_tensor("xt", [K, npad], mybir.dt.float32, kind="ExternalInput")
    w_d = nc.dram_tensor("w", [K, F], mybir.dt.float32, kind="ExternalInput")
    dinv_d = nc.dram_tensor(
        "dinv", [128, npad // 128], mybir.dt.float32, kind="ExternalInput"
    )
    g_d = nc.dram_tensor("g", [npad, F], mybir.dt.float32, kind="ExternalOutput")

    with tile.TileContext(nc) as tc:
        with (
            tc.tile_pool(name="const", bufs=1) as cpool,
            tc.tile_pool(name="xin", bufs=3) as xpool,
            tc.tile_pool(name="gout", bufs=3) as gpool,
            tc.tile_pool(name="ph", bufs=4, space="PSUM") as php,
        ):
            w_t = cpool.tile([K, F], mybir.dt.float32)
            nc.sync.dma_start(out=w_t[:], in_=w_d[:])
            dinv_t = cpool.tile([128, npad // 128], mybir.dt.float32)
            nc.sync.dma_start(out=dinv_t[:], in_=dinv_d[:])

            for t in range(T):
                xT_t = xpool.tile([K, 512], mybir.dt.float32, tag="xT")
                nc.sync.dma_start(out=xT_t[:], in_=xt_d[:, t * 512 : (t + 1) * 512])
                g4_t = gpool.tile([128, 4, F], mybir.dt.float32, tag="g4")
                for a in range(4):
                    h_p = php.tile([128, F], mybir.dt.float32, tag="h")
                    nc.tensor.matmul(
                        h_p[:],
                        xT_t[:, a * 128 : (a + 1) * 128],
                        w_t[:],
                        start=True,
                        stop=True,
                    )
                    nc.vector.tensor_scalar_mul(
                        out=g4_t[:, a, :],
                        in0=h_p[:],
                        scalar1=dinv_t[:, 4 * t + a : 4 * t + a + 1],
                    )
                nc.sync.dma_start(
                    out=g_d[t * 512 : (t + 1) * 512, :].rearrange(
                        "(a p) f -> p a f", a=4
                    ),
                    in_=g4_t[:],
                )
    nc.compile()
    return nc


# ---------------------------------------------------------------- launch B


def build_launch_b(cfg: Cfg, batches):
    nc = bacc.Bacc(
        "TRN2", target_bir_lowering=False, debug=False, num_devices=cfg.n_cores
    )
    NI, NB = cfg.ni, len(batches)

    tab_d = nc.dram_tensor(
        "tab", [128, cfg.elems], mybir.dt.float32, kind="ExternalInput"
    )
    idx_d = nc.dram_tensor(
        "idx", [NB, 128, NI // 16], mybir.dt.int16, kind="ExternalInput"
    )
    acc_d = nc.dram_tensor(
        "acc", [cfg.n_q, 128, cfg.nloc], mybir.dt.bfloat16, kind="ExternalOutput"
    )

    with tile.TileContext(nc) as tc:
        with (
            tc.tile_pool(name="tabp", bufs=1) as tpool,
            tc.tile_pool(name="accp", bufs=1) as apool,
            tc.tile_pool(name="idxp", bufs=3) as ipool,
            tc.tile_pool(name="msgp", bufs=2) as mpool,
        ):
            tab_t = tpool.tile([128, cfg.elems], mybir.dt.float32)
            # load per-quarter so the first gather only waits for its slice
            for q in range(cfg.n_q):
                sl = slice(q * cfg.qelems, (q + 1) * cfg.qelems)
                nc.sync.dma_start(out=tab_t[:, sl], in_=tab_d[:, sl])
            acc_t = apool.tile([128, cfg.nloc], mybir.dt.bfloat16)

            for b, (q, segs) in enumerate(batches):
                idx_t = ipool.tile([128, NI // 16], mybir.dt.int16, tag="i")
                nc.sync.dma_start(out=idx_t[:], in_=idx_d[b])
                msg_t = mpool.tile([128, NI], mybir.dt.float32, tag="m")
                nc.gpsimd.ap_gather(
                    msg_t[:].unsqueeze(2),
                    tab_t[:, q * cfg.qelems : (q + 1) * cfg.qelems].unsqueeze(2),
                    idx_t[:],
                    channels=128,
                    num_elems=cfg.qelems,
                    d=1,
                    num_idxs=NI,
                )
                # bf16 accumulator is deliberate: one rounded write per dest;
                # validated rel err ~4e-3 vs the 2e-2 budget.
                with nc.allow_low_precision(reason="bf16 acc validated"):
                    for r0, m, k, off in segs:
                        nc.vector.tensor_reduce(
                            out=acc_t[:, r0 : r0 + m].unsqueeze(2),
                            in_=msg_t[:, off : off + m * k].rearrange(
                                "p (m k) -> p m k", m=m
                            ),
                            op=mybir.AluOpType.add,
                            axis=mybir.AxisListType.X,
                        )
                if b + 1 == NB or batches[b + 1][0] != q:
                    nc.sync.dma_start(out=acc_d[q], in_=acc_t[:])
    nc.compile()
    return nc


# ---------------------------------------------------------------- launch C


def build_launch_c(cfg: Cfg):
    nc = bacc.Bacc(
        "TRN2", target_bir_lowering=False, debug=False, num_devices=cfg.n_cores
    )
    F, EC = cfg.net_dim, cfg.echunk
    n_chunks = -(-cfg.nloc // EC)

    h_d = nc.dram_tensor(
        "hcat", [cfg.n_q, 128, cfg.nloc], mybir.dt.bfloat16, kind="ExternalInput"
    )
    rmat_d = nc.dram_tensor("rmat", [128, F], mybir.dt.bfloat16, kind="ExternalInput")
    dinv_d = nc.dram_tensor(
        "dinv", [F, cfg.nloc], mybir.dt.float32, kind="ExternalInput"
    )
    bg_d = nc.dram_tensor("bg", [F, 1], mybir.dt.float32, kind="ExternalInput")
    wd_d = nc.dram_tensor("wd", [F, F], mybir.dt.bfloat16, kind="ExternalInput")
    bd_d = nc.dram_tensor("bd", [F, 1], mybir.dt.float32, kind="ExternalInput")
    out_d = nc.dram_tensor(
        "out", [F, cfg.nloc], mybir.dt.float32, kind="ExternalOutput"
    )

    with tile.TileContext(nc) as tc:
        with (
            tc.tile_pool(name="const", bufs=1) as cpool,
            tc.tile_pool(name="hin", bufs=1) as hpool,
            tc.tile_pool(name="work", bufs=2) as wpool,
            tc.tile_pool(name="p1", bufs=2, space="PSUM") as p1p,
            tc.tile_pool(name="p2", bufs=2, space="PSUM") as p2p,
        ):
            rmat_t = cpool.tile([128, F], mybir.dt.bfloat16)
            nc.sync.dma_start(out=rmat_t[:], in_=rmat_d[:])
            dinv_t = cpool.tile([F, cfg.nloc], mybir.dt.float32)
            nc.sync.dma_start(out=dinv_t[:], in_=dinv_d[:])
            bg_t = cpool.tile([F, 1], mybir.dt.float32)
            nc.sync.dma_start(out=bg_t[:], in_=bg_d[:])
            wd_t = cpool.tile([F, F], mybir.dt.bfloat16)
            nc.sync.dma_start(out=wd_t[:], in_=wd_d[:])
            bd_t = cpool.tile([F, 1], mybir.dt.float32)
            nc.sync.dma_start(out=bd_t[:], in_=bd_d[:])
            h_t = hpool.tile([128, cfg.n_q, cfg.nloc], mybir.dt.bfloat16)
            for q in range(cfg.n_q):
                nc.sync.dma_start(out=h_t[:, q, :], in_=h_d[q])

            for j in range(n_chunks):
                c0 = j * EC
                w = min(EC, cfg.nloc - c0)
                h1_p = p1p.tile([F, EC], mybir.dt.float32, tag="h1")
                for q in range(cfg.n_q):
                    nc.tensor.matmul(
                        h1_p[:, :w],
                        rmat_t[:],
                        h_t[:, q, c0 : c0 + w],
                        start=(q == 0),
                        stop=(q == cfg.n_q - 1),
                    )
                h1_t = wpool.tile([F, EC], mybir.dt.float32, tag="h1s")
                nc.vector.tensor_tensor(
                    out=h1_t[:, :w],
                    in0=h1_p[:, :w],
                    in1=dinv_t[:, c0 : c0 + w],
                    op=mybir.AluOpType.mult,
                )
                h1r_t = wpool.tile([F, EC], mybir.dt.bfloat16, tag="h1r")
                nc.scalar.activation(
                    h1r_t[:, :w],
                    h1_t[:, :w],
                    mybir.ActivationFunctionType.Relu,
                    bias=bg_t[:],
                )
                h2_p = p2p.tile([F, EC], mybir.dt.float32, tag="h2")
                nc.tensor.matmul(h2_p[:, :w], wd_t[:], h1r_t[:, :w], start=True, stop=True)
                o_t = wpool.tile([F, EC], mybir.dt.float32, tag="o")
                nc.scalar.activation(
                    o_t[:, :w],
                    h2_p[:, :w],
                    mybir.ActivationFunctionType.Relu,
                    bias=bd_t[:],
                )
                nc.sync.dma_start(out=out_d[:, c0 : c0 + w], in_=o_t[:, :w])
    nc.compile()
    return nc


# ---------------------------------------------------------------- host side


def host_prep(x, edge_index, W_gcn, b_gcn, W_dense, b_dense, cfg: Cfg):
    n, nloc, SUB, QS = cfg.n, cfg.nloc, cfg.sub, cfg.qsub
    row = np.asarray(edge_index[0], dtype=np.int64)
    col = np.asarray(edge_index[1], dtype=np.int64)
    deg = np.bincount(col, minlength=n).astype(np.int64) + 1  # + self-loop
    dinv = (1.0 / np.sqrt(deg)).astype(np.float32)

    W_gcn = _f32(W_gcn)
    b_gcn = _f32(b_gcn).reshape(cfg.net_dim, 1)
    wd = _bf16(W_dense)
    bd = _f32(b_dense).reshape(cfg.net_dim, 1)
    xT = np.ascontiguousarray(_f32(x).T)  # [128, n]
    npad = -(-nloc // 512) * 512

    # per-core edges (dest-owner) with self-loops appended
    owner = col // nloc
    srcs_c, dstl_c = [], []
    for c in range(cfg.n_cores):
        m = owner == c
        loop = np.arange(c * nloc, (c + 1) * nloc, dtype=np.int64)
        srcs_c.append(np.concatenate([row[m], loop]))
        dstl_c.append(np.concatenate([col[m] - c * nloc, loop - c * nloc]))

    # per-(core, stream, quarter) in-degree and rank permutations
    sdeg = np.zeros((cfg.n_cores, cfg.n_sub, cfg.n_q, nloc), dtype=np.int64)
    for c in range(cfg.n_cores):
        st = srcs_c[c] // SUB
        qt = (srcs_c[c] % SUB) // QS
        for s in range(cfg.n_sub):
            for q in range(cfg.n_q):
                sdeg[c, s, q] = np.bincount(
                    dstl_c[c][(st == s) & (qt == q)], minlength=nloc
                )
    perms = np.argsort(-sdeg, axis=3, kind="stable")
    sorted_deg = np.take_along_axis(sdeg, perms, axis=3)
    kreq = np.maximum(sorted_deg.max(axis=(0, 1)), 1)  # [n_q, nloc]
    batches = build_schedule(kreq, cfg.ni)
    NB = len(batches)

    # slot base offsets per (quarter, rank) within that quarter's batches;
    # slot index is (batch number, offset) -> flat NB*ni slot space.
    slot_base = np.zeros((cfg.n_q, nloc), dtype=np.int64)
    for b, (q, segs) in enumerate(batches):
        for r0, m, k, off in segs:
            slot_base[q, r0 : r0 + m] = b * cfg.ni + off + np.arange(m) * k

    rmat = np.zeros((128, cfg.net_dim), dtype=np.float32)
    rmat[np.arange(128), np.arange(128) % 32] = 1.0

    in_a, in_b, in_c = [], [], []
    for c in range(cfg.n_cores):
        xpad = np.zeros((cfg.in_dim, npad), dtype=np.float32)
        xpad[:, :nloc] = xT[:, c * nloc : (c + 1) * nloc]
        dpad = np.ones(npad, dtype=np.float32)
        dpad[:nloc] = dinv[c * nloc : (c + 1) * nloc]
        dinv_a = np.ascontiguousarray(dpad.reshape(npad // 128, 128).T)
        in_a.append({"xt": xpad, "w": W_gcn, "dinv": dinv_a})

        # slot fill: idx_all[s] is the flat slot space for stream s
        idx_all = np.full((cfg.n_sub, NB * cfg.ni), QS, dtype=np.int64)
        st = srcs_c[c] // SUB
        qt = (srcs_c[c] % SUB) // QS
        for s in range(cfg.n_sub):
            for q in range(cfg.n_q):
                m = (st == s) & (qt == q)
                es = (srcs_c[c][m] % SUB) - q * QS  # idx within quarter slice
                ed = dstl_c[c][m]
                rank_of = np.empty(nloc, dtype=np.int64)
                rank_of[perms[c, s, q]] = np.arange(nloc)
                er = rank_of[ed]
                order = np.argsort(er, kind="stable")
                ero, eso = er[order], es[order]
                uq, starts, counts = np.unique(
                    ero, return_index=True, return_counts=True
                )
                occ = np.arange(ero.size) - np.repeat(starts, counts)
                idx_all[s][slot_base[q][ero] + occ] = eso
        # wrap to [NB, 128, ni/16]: partition p serves stream p//32; its
        # idx j sits at (row p%16, col j//16) per the interp unwrap order.
        wrapped = (
            idx_all.reshape(cfg.n_sub, NB, cfg.ni // 16, 16)
            .transpose(1, 0, 3, 2)
            .astype(np.int16)
        )  # [NB, n_sub, 16, ni/16]
        stream_of = np.arange(128) // 32
        part16 = np.arange(128) % 16
        idx_tiles = np.ascontiguousarray(wrapped[:, stream_of, part16, :])
        in_b.append({"tab": None, "idx": idx_tiles})

        dinv_rank = np.ascontiguousarray(
            np.broadcast_to(dinv[c * nloc : (c + 1) * nloc], (cfg.net_dim, nloc))
        )
        in_c.append(
            {
                "hcat": None,
                "rmat": _bf16(rmat),
                "dinv": dinv_rank,
                "bg": b_gcn,
                "wd": wd,
                "bd": bd,
            }
        )
    return in_a, in_b, in_c, batches, perms


def assemble_table(res_a, cfg: Cfg):
    g = np.concatenate(
        [res_a[c]["g"][: cfg.nloc] for c in range(cfg.n_cores)], axis=0
    )  # [n, 32] f32
    tab = np.zeros((128, cfg.elems), dtype=np.float32)
    # tab[32s+f, qelems*q + j] = g[25000 s + 6250 q + j, f]
    t = g.reshape(cfg.n_sub, cfg.n_q, cfg.qsub, cfg.net_dim)
    for q in range(cfg.n_q):
        blk = t[:, q].transpose(0, 2, 1).reshape(128, cfg.qsub)
        tab[:, q * cfg.qelems : q * cfg.qelems + cfg.qsub] = blk
    return tab


def align_acc(res_b, perms, cfg: Cfg):
    """acc [n_q, 128, nloc] bf16 per core, per-(stream, quarter) rank
    order -> canonical dest order."""
    hcats = []
    for c in range(cfg.n_cores):
        acc = np.asarray(res_b[c]["acc"])
        hcat = np.empty_like(acc)
        for q in range(cfg.n_q):
            for s in range(cfg.n_sub):
                blk = acc[q, 32 * s : 32 * s + 32]
                hcat[q, 32 * s : 32 * s + 32][:, perms[c, s, q]] = blk
        hcats.append(hcat)
    return hcats


def assemble_out(res_c, cfg: Cfg):
    return np.concatenate(
        [np.asarray(res_c[c]["out"], dtype=np.float32).T for c in range(cfg.n_cores)],
        axis=0,
    )


_NC_CACHE = {}


def _get_ncs(cfg: Cfg, batches):
    key = tuple(batches)
    if key not in _NC_CACHE:
        _NC_CACHE.clear()
        _NC_CACHE[key] = (
            build_launch_a(cfg),
            build_launch_b(cfg, batches),
            build_launch_c(cfg),
        )
    return _NC_CACHE[key]


def kernel(x, edge_index, W_gcn, b_gcn, W_dense, b_dense):
    cfg = FULL
    in_a, in_b, in_c, batches, perms = host_prep(
        x, edge_index, W_gcn, b_gcn, W_dense, b_dense, cfg
    )
    nc_a, nc_b, nc_c = _get_ncs(cfg, batches)
    core_ids = list(range(cfg.n_cores))
    res_a = run_bass_kernel_spmd(nc_a, in_a, core_ids).results
    tab = assemble_table(res_a, cfg)
    for m in in_b:
        m["tab"] = tab
    res_b = run_bass_kernel_spmd(nc_b, in_b, core_ids).results
    hcats = align_acc(res_b, perms, cfg)
    for c, m in enumerate(in_c):
        m["hcat"] = _bf16(hcats[c])
    res_c = run_bass_kernel_spmd(nc_c, in_c, core_ids).results
    return assemble_out(res_c, cfg)


# revision 35
# speedup vs baseline: 1.3415x; 1.3415x over previous
"""GCN layer (GCNConv + relu + dense + relu) on 8 Trainium2 NeuronCores.

Strategy (v3.2: SBUF table + gpsimd ap_gather + shared-shape reduce)
-------------------------------------------------------------------
Math: out = relu(relu(GCNConv(x)) @ W_dense + b_dense) with
GCNConv(x)[v] = dinv[v] * sum_{e: s->v} dinv[s] * (x W_gcn)[s] + b_gcn
(self-loops as ordinary edges; dinv = rsqrt(indegree incl. self).)

Launch A (per core): g = dinv[node] * (x @ W_gcn), f32 node-major.
Host reassembles the global feature table
tab[32*s + f, 6251*q + j] = g[25000*s + 6250*q + j, f]  ([128, 25004]
f32) — partitions carry (source-subset s, feature f); each of the 4
quarters ends with a zero element (index 6250) used for slot padding.

Launch B (per core): the table lives in SBUF.  Four independent
"streams" (gpsimd core pairs, partitions [32s, 32s+32)) gather their
source subset with ONE ap_gather per batch.  Work is phased by source
QUARTER, so each instruction's table slice is only 6251 elements (the
gpsimd cost scales with the table operand).  Destinations are sorted
by per-(stream, quarter) in-degree; all 4 streams share each batch's
(m dests x K slots) shape, so a single [128, m, K] tensor_reduce sums
4 streams' dest runs at once into a bf16 accumulator that is flushed
to HBM after each quarter.  No mask, no per-edge DMA, no scatter.

Host re-aligns the 16 (stream, quarter) rank orders to canonical dest
order (pure permutation).  Launch C: 4 accumulating PE matmuls with an
indicator lhsT collapse quarters and subsets, then * dinv[dest],
+b_gcn, relu, @W_dense, +b_dense, relu -> [32, 12500] f32 per core.
Host transposes.
"""

import sys

if "/opt/trn_rl_repo" not in sys.path:
    sys.path.insert(0, "/opt/trn_rl_repo")

from dataclasses import dataclass

import ml_dtypes
import numpy as np

import concourse.bacc as bacc
import concourse.mybir as mybir
from concourse import tile
from concourse.bass_utils import run_bass_kernel_spmd

BF16 = ml_dtypes.bfloat16


@dataclass(frozen=True)
class Cfg:
    n_cores: int = 8
    nloc: int = 12500
    in_dim: int = 128
    net_dim: int = 32
    n_sub: int = 4            # source subsets = streams (32 partitions each)
    n_q: int = 4              # source quarters within a subset
    ni: int = 8192            # gather idxs per instruction per core section
    echunk: int = 512         # launch-C column chunk (one PSUM bank)

    @property
    def n(self):
        return self.nloc * self.n_cores

    @property
    def sub(self):
        return self.n // self.n_sub          # 25000 nodes per subset

    @property
    def qsub(self):
        return self.sub // self.n_q          # 6250 nodes per quarter

    @property
    def qelems(self):
        return self.qsub + 1                 # +1 zero element per quarter

    @property
    def elems(self):
        return self.n_q * self.qelems        # 25004 table columns


FULL = Cfg()
assert FULL.n == 100000 and FULL.sub == 25000 and FULL.qsub == 6250


def _f32(x):
    return np.ascontiguousarray(x, dtype=np.float32)


def _bf16(x):
    return np.ascontiguousarray(x, dtype=BF16)


# ------------------------------------------------------------ schedule


def build_schedule(kreq, ni):
    """kreq: [n_q, nloc], per-quarter per-rank slot needs (descending,
    shared by all streams and cores).  Returns batches [(q, segs)] where
    segs = ((r0, m, K, off), ...): each segment is a run of m dests with
    exactly K slots each, at slot offset `off` within the batch."""
    n_q, nloc = kreq.shape
    batches = []
    for q in range(n_q):
        kr = np.maximum(kreq[q], 1).astype(np.int64)
        segs, used = [], 0
        r = 0
        while r < nloc:
            k = int(kr[r])
            assert k <= ni
            r_end = r + int(np.searchsorted(-kr[r:], -k, side="right"))
            while r < r_end:
                cap = (ni - used) // k
                if cap == 0:
                    batches.append((q, tuple(segs)))
                    segs, used = [], 0
                    cap = ni // k
                m = min(cap, r_end - r)
                segs.append((r, m, k, used))
                used += m * k
                r += m
        if segs:
            batches.append((q, tuple(segs)))
    return batches


# ---------------------------------------------------------------- launch A


def build_launch_a(cfg: Cfg):
    nc = bacc.Bacc(
        "TRN2", target_bir_lowering=False, debug=False, num_devices=cfg.n_cores
    )
    K, F = cfg.in_dim, cfg.net_dim
    npad = -(-cfg.nloc // 1280) * 1280  # 12800
    T = npad // 1280
    xt_d = nc.dram_tensor("xt", [K, npad], mybir.dt.bfloat16, kind="ExternalInput")
    w_d = nc.dram_tensor("w", [K, F], mybir.dt.bfloat16, kind="ExternalInput")
    dinv_d = nc.dram_tensor(
        "dinv", [128, npad // 128], mybir.dt.float32, kind="ExternalInput"
    )
    g_d = nc.dram_tensor("g", [npad, F], mybir.dt.float32, kind="ExternalOutput")

    with tile.TileContext(nc) as tc:
        with (
            tc.tile_pool(name="const", bufs=1) as cpool,
            tc.tile_pool(name="xin", bufs=3) as xpool,
            tc.tile_pool(name="gout", bufs=3) as gpool,
            tc.tile_pool(name="ph", bufs=4, space="PSUM") as php,
        ):
            w_t = cpool.tile([K, F], mybir.dt.bfloat16)
            nc.sync.dma_start(out=w_t[:], in_=w_d[:])
            dinv_t = cpool.tile([128, npad // 128], mybir.dt.float32)
            nc.sync.dma_start(out=dinv_t[:], in_=dinv_d[:])

            for t in range(T):
                xT_t = xpool.tile([K, 1280], mybir.dt.bfloat16, tag="xT")
                nc.sync.dma_start(out=xT_t[:], in_=xt_d[:, t * 1280 : (t + 1) * 1280])
                g4_t = gpool.tile([128, 10, F], mybir.dt.float32, tag="g4")
                for a in range(10):
                    h_p = php.tile([128, F], mybir.dt.float32, tag="h")
                    nc.tensor.matmul(
                        h_p[:],
                        xT_t[:, a * 128 : (a + 1) * 128],
                        w_t[:],
                        start=True,
                        stop=True,
                    )
                    nc.vector.tensor_scalar_mul(
                        out=g4_t[:, a, :],
                        in0=h_p[:],
                        scalar1=dinv_t[:, 10 * t + a : 10 * t + a + 1],
                    )
                nc.sync.dma_start(
                    out=g_d[t * 1280 : (t + 1) * 1280, :].rearrange(
                        "(a p) f -> p a f", a=10
                    ),
                    in_=g4_t[:],
                )
    nc.compile()
    return nc


# ---------------------------------------------------------------- launch B


def build_launch_b(cfg: Cfg, batches):
    nc = bacc.Bacc(
        "TRN2", target_bir_lowering=False, debug=False, num_devices=cfg.n_cores
    )
    NI, NB = cfg.ni, len(batches)

    tab_d = nc.dram_tensor(
        "tab", [128, cfg.elems], mybir.dt.float32, kind="ExternalInput"
    )
    idx_d = nc.dram_tensor(
        "idx", [NB, 128, NI // 16], mybir.dt.int16, kind="ExternalInput"
    )
    acc_d = nc.dram_tensor(
        "acc", [cfg.n_q, 128, cfg.nloc], mybir.dt.bfloat16, kind="ExternalOutput"
    )

    with tile.TileContext(nc) as tc:
        with (
            tc.tile_pool(name="tabp", bufs=1) as tpool,
            tc.tile_pool(name="accp", bufs=1) as apool,
            tc.tile_pool(name="idxp", bufs=3) as ipool,
            tc.tile_pool(name="msgp", bufs=2) as mpool,
        ):
            # one tile per quarter; each stripe is loaded just before its
            # quarter's first batch so stripes 1-3 hide behind compute
            tab_t = [
                tpool.tile([128, cfg.qelems], mybir.dt.float32, name=f"tab{q}")
                for q in range(cfg.n_q)
            ]
            loaded = set()
            acc_t = apool.tile([128, cfg.nloc], mybir.dt.bfloat16)

            for b, (q, segs) in enumerate(batches):
                if q not in loaded:
                    loaded.add(q)
                    sl = slice(q * cfg.qelems, (q + 1) * cfg.qelems)
                    nc.sync.dma_start(out=tab_t[q][:], in_=tab_d[:, sl])
                idx_t = ipool.tile([128, NI // 16], mybir.dt.int16, tag="i")
                nc.sync.dma_start(out=idx_t[:], in_=idx_d[b])
                msg_t = mpool.tile([128, NI], mybir.dt.float32, tag="m")
                nc.gpsimd.ap_gather(
                    msg_t[:].unsqueeze(2),
                    tab_t[q][:].unsqueeze(2),
                    idx_t[:],
                    channels=128,
                    num_elems=cfg.qelems,
                    d=1,
                    num_idxs=NI,
                )
                # bf16 accumulator is deliberate: one rounded write per dest;
                # validated rel err ~4e-3 vs the 2e-2 budget.
                with nc.allow_low_precision(reason="bf16 acc validated"):
                    for r0, m, k, off in segs:
                        nc.vector.tensor_reduce(
                            out=acc_t[:, r0 : r0 + m].unsqueeze(2),
                            in_=msg_t[:, off : off + m * k].rearrange(
                                "p (m k) -> p m k", m=m
                            ),
                            op=mybir.AluOpType.add,
                            axis=mybir.AxisListType.X,
                        )
                # flush this batch's freshly-written acc column slice; keeps
                # the WAR window for the next quarter to a single batch span
                lo = min(r0 for r0, _, _, _ in segs)
                hi = max(r0 + m for r0, m, _, _ in segs)
                nc.sync.dma_start(out=acc_d[q, :, lo:hi], in_=acc_t[:, lo:hi])
    nc.compile()
    return nc


# ---------------------------------------------------------------- launch C


def build_launch_c(cfg: Cfg):
    nc = bacc.Bacc(
        "TRN2", target_bir_lowering=False, debug=False, num_devices=cfg.n_cores
    )
    F, EC = cfg.net_dim, cfg.echunk
    n_chunks = -(-cfg.nloc // EC)

    h_d = nc.dram_tensor(
        "hcat", [cfg.n_q, 128, cfg.nloc], mybir.dt.bfloat16, kind="ExternalInput"
    )
    rmat_d = nc.dram_tensor("rmat", [128, F], mybir.dt.bfloat16, kind="ExternalInput")
    dinv_d = nc.dram_tensor(
        "dinv", [F, cfg.nloc], mybir.dt.bfloat16, kind="ExternalInput"
    )
    bg_d = nc.dram_tensor("bg", [F, 1], mybir.dt.float32, kind="ExternalInput")
    wd_d = nc.dram_tensor("wd", [F, F], mybir.dt.bfloat16, kind="ExternalInput")
    bd_d = nc.dram_tensor("bd", [F, 1], mybir.dt.float32, kind="ExternalInput")
    out_d = nc.dram_tensor(
        "out", [F, cfg.nloc], mybir.dt.float32, kind="ExternalOutput"
    )

    with tile.TileContext(nc) as tc:
        with (
            tc.tile_pool(name="const", bufs=1) as cpool,
            tc.tile_pool(name="hin", bufs=1) as hpool,
            tc.tile_pool(name="work", bufs=2) as wpool,
            tc.tile_pool(name="p1", bufs=2, space="PSUM") as p1p,
            tc.tile_pool(name="p2", bufs=2, space="PSUM") as p2p,
        ):
            rmat_t = cpool.tile([128, F], mybir.dt.bfloat16)
            nc.sync.dma_start(out=rmat_t[:], in_=rmat_d[:])
            dinv_t = cpool.tile([F, cfg.nloc], mybir.dt.bfloat16)
            nc.sync.dma_start(out=dinv_t[:], in_=dinv_d[:])
            bg_t = cpool.tile([F, 1], mybir.dt.float32)
            nc.sync.dma_start(out=bg_t[:], in_=bg_d[:])
            wd_t = cpool.tile([F, F], mybir.dt.bfloat16)
            nc.sync.dma_start(out=wd_t[:], in_=wd_d[:])
            bd_t = cpool.tile([F, 1], mybir.dt.float32)
            nc.sync.dma_start(out=bd_t[:], in_=bd_d[:])
            # hcat loads in column slabs, emitted just ahead of first use so
            # they interleave with out-writes on the DMA engines
            SLAB = 2 * EC
            h_t = hpool.tile([128, cfg.n_q, cfg.nloc], mybir.dt.bfloat16)

            def load_slab(s0):
                sw = min(SLAB, cfg.nloc - s0)
                nc.sync.dma_start(
                    out=h_t[:, :, s0 : s0 + sw],
                    in_=h_d[:, :, s0 : s0 + sw].rearrange("q p w -> p q w"),
                )

            load_slab(0)
            for j in range(n_chunks):
                c0 = j * EC
                w = min(EC, cfg.nloc - c0)
                nxt = c0 + 2 * EC  # prefetch one slab ahead
                if nxt % SLAB == 0 and nxt < cfg.nloc:
                    load_slab(nxt)
                h1_p = p1p.tile([F, EC], mybir.dt.float32, tag="h1")
                for q in range(cfg.n_q):
                    nc.tensor.matmul(
                        h1_p[:, :w],
                        rmat_t[:],
                        h_t[:, q, c0 : c0 + w],
                        start=(q == 0),
                        stop=(q == cfg.n_q - 1),
                    )
                h1_t = wpool.tile([F, EC], mybir.dt.float32, tag="h1s")
                nc.vector.tensor_tensor(
                    out=h1_t[:, :w],
                    in0=h1_p[:, :w],
                    in1=dinv_t[:, c0 : c0 + w],
                    op=mybir.AluOpType.mult,
                )
                h1r_t = wpool.tile([F, EC], mybir.dt.bfloat16, tag="h1r")
                nc.scalar.activation(
                    h1r_t[:, :w],
                    h1_t[:, :w],
                    mybir.ActivationFunctionType.Relu,
                    bias=bg_t[:],
                )
                h2_p = p2p.tile([F, EC], mybir.dt.float32, tag="h2")
                nc.tensor.matmul(h2_p[:, :w], wd_t[:], h1r_t[:, :w], start=True, stop=True)
                o_t = wpool.tile([F, EC], mybir.dt.float32, tag="o")
                nc.scalar.activation(
                    o_t[:, :w],
                    h2_p[:, :w],
                    mybir.ActivationFunctionType.Relu,
                    bias=bd_t[:],
                )
                nc.sync.dma_start(out=out_d[:, c0 : c0 + w], in_=o_t[:, :w])
    nc.compile()
    return nc


# ---------------------------------------------------------------- host side


def host_prep(x, edge_index, W_gcn, b_gcn, W_dense, b_dense, cfg: Cfg):
    n, nloc, SUB, QS = cfg.n, cfg.nloc, cfg.sub, cfg.qsub
    row = np.asarray(edge_index[0], dtype=np.int64)
    col = np.asarray(edge_index[1], dtype=np.int64)
    deg = np.bincount(col, minlength=n).astype(np.int64) + 1  # + self-loop
    dinv = (1.0 / np.sqrt(deg)).astype(np.float32)

    W_gcn = _bf16(W_gcn)
    b_gcn = _f32(b_gcn).reshape(cfg.net_dim, 1)
    wd = _bf16(W_dense)
    bd = _f32(b_dense).reshape(cfg.net_dim, 1)
    xT = np.ascontiguousarray(_f32(x).T)  # [128, n]
    npad = -(-nloc // 1280) * 1280

    # per-core edges (dest-owner) with self-loops appended
    owner = col // nloc
    srcs_c, dstl_c = [], []
    for c in range(cfg.n_cores):
        m = owner == c
        loop = np.arange(c * nloc, (c + 1) * nloc, dtype=np.int64)
        srcs_c.append(np.concatenate([row[m], loop]))
        dstl_c.append(np.concatenate([col[m] - c * nloc, loop - c * nloc]))

    # per-(core, stream, quarter) in-degree and rank permutations
    sdeg = np.zeros((cfg.n_cores, cfg.n_sub, cfg.n_q, nloc), dtype=np.int64)
    for c in range(cfg.n_cores):
        st = srcs_c[c] // SUB
        qt = (srcs_c[c] % SUB) // QS
        for s in range(cfg.n_sub):
            for q in range(cfg.n_q):
                sdeg[c, s, q] = np.bincount(
                    dstl_c[c][(st == s) & (qt == q)], minlength=nloc
                )
    perms = np.argsort(-sdeg, axis=3, kind="stable")
    sorted_deg = np.take_along_axis(sdeg, perms, axis=3)
    kreq = np.maximum(sorted_deg.max(axis=(0, 1)), 1)  # [n_q, nloc]
    batches = build_schedule(kreq, cfg.ni)
    NB = len(batches)

    # slot base offsets per (quarter, rank) within that quarter's batches;
    # slot index is (batch number, offset) -> flat NB*ni slot space.
    slot_base = np.zeros((cfg.n_q, nloc), dtype=np.int64)
    for b, (q, segs) in enumerate(batches):
        for r0, m, k, off in segs:
            slot_base[q, r0 : r0 + m] = b * cfg.ni + off + np.arange(m) * k

    rmat = np.zeros((128, cfg.net_dim), dtype=np.float32)
    rmat[np.arange(128), np.arange(128) % 32] = 1.0

    in_a, in_b, in_c = [], [], []
    for c in range(cfg.n_cores):
        xpad = np.zeros((cfg.in_dim, npad), dtype=BF16)
        xpad[:, :nloc] = xT[:, c * nloc : (c + 1) * nloc].astype(BF16)
        dpad = np.ones(npad, dtype=np.float32)
        dpad[:nloc] = dinv[c * nloc : (c + 1) * nloc]
        dinv_a = np.ascontiguousarray(dpad.reshape(npad // 128, 128).T)
        in_a.append({"xt": xpad, "w": W_gcn, "dinv": dinv_a})

        # slot fill: idx_all[s] is the flat slot space for stream s
        idx_all = np.full((cfg.n_sub, NB * cfg.ni), QS, dtype=np.int64)
        st = srcs_c[c] // SUB
        qt = (srcs_c[c] % SUB) // QS
        for s in range(cfg.n_sub):
            for q in range(cfg.n_q):
                m = (st == s) & (qt == q)
                es = (srcs_c[c][m] % SUB) - q * QS  # idx within quarter slice
                ed = dstl_c[c][m]
                rank_of = np.empty(nloc, dtype=np.int64)
                rank_of[perms[c, s, q]] = np.arange(nloc)
                er = rank_of[ed]
                order = np.argsort(er, kind="stable")
                ero, eso = er[order], es[order]
                uq, starts, counts = np.unique(
                    ero, return_index=True, return_counts=True
                )
                occ = np.arange(ero.size) - np.repeat(starts, counts)
                idx_all[s][slot_base[q][ero] + occ] = eso
        # wrap to [NB, 128, ni/16]: partition p serves stream p//32; its
        # idx j sits at (row p%16, col j//16) per the interp unwrap order.
        wrapped = (
            idx_all.reshape(cfg.n_sub, NB, cfg.ni // 16, 16)
            .transpose(1, 0, 3, 2)
            .astype(np.int16)
        )  # [NB, n_sub, 16, ni/16]
        stream_of = np.arange(128) // 32
        part16 = np.arange(128) % 16
        idx_tiles = np.ascontiguousarray(wrapped[:, stream_of, part16, :])
        in_b.append({"tab": None, "idx": idx_tiles})

        dinv_rank = _bf16(
            np.broadcast_to(dinv[c * nloc : (c + 1) * nloc], (cfg.net_dim, nloc))
        )
        in_c.append(
            {
                "hcat": None,
                "rmat": _bf16(rmat),
                "dinv": dinv_rank,
                "bg": b_gcn,
                "wd": wd,
                "bd": bd,
            }
        )
    return in_a, in_b, in_c, batches, perms


def assemble_table(res_a, cfg: Cfg):
    g = np.concatenate(
        [res_a[c]["g"][: cfg.nloc] for c in range(cfg.n_cores)], axis=0
    )  # [n, 32] f32
    tab = np.zeros((128, cfg.elems), dtype=np.float32)
    # tab[32s+f, qelems*q + j] = g[25000 s + 6250 q + j, f]
    t = g.reshape(cfg.n_sub, cfg.n_q, cfg.qsub, cfg.net_dim)
    for q in range(cfg.n_q):
        blk = t[:, q].transpose(0, 2, 1).reshape(128, cfg.qsub)
        tab[:, q * cfg.qelems : q * cfg.qelems + cfg.qsub] = blk
    return tab


def align_acc(res_b, perms, cfg: Cfg):
    """acc [n_q, 128, nloc] bf16 per core, per-(stream, quarter) rank
    order -> canonical dest order."""
    hcats = []
    for c in range(cfg.n_cores):
        acc = np.asarray(res_b[c]["acc"])
        hcat = np.empty_like(acc)
        for q in range(cfg.n_q):
            for s in range(cfg.n_sub):
                blk = acc[q, 32 * s : 32 * s + 32]
                hcat[q, 32 * s : 32 * s + 32][:, perms[c, s, q]] = blk
        hcats.append(hcat)
    return hcats


def assemble_out(res_c, cfg: Cfg):
    return np.concatenate(
        [np.asarray(res_c[c]["out"], dtype=np.float32).T for c in range(cfg.n_cores)],
        axis=0,
    )


_NC_CACHE = {}


def _get_ncs(cfg: Cfg, batches):
    key = tuple(batches)
    if key not in _NC_CACHE:
        _NC_CACHE.clear()
        _NC_CACHE[key] = (
            build_launch_a(cfg),
            build_launch_b(cfg, batches),
            build_launch_c(cfg),
        )
    return _NC_CACHE[key]


def kernel(x, edge_index, W_gcn, b_gcn, W_dense, b_dense):
    cfg = FULL
    in_a, in_b, in_c, batches, perms = host_prep(
        x, edge_index, W_gcn, b_gcn, W_dense, b_dense, cfg
    )
    nc_a, nc_b, nc_c = _get_ncs(cfg, batches)
    core_ids = list(range(cfg.n_cores))
    res_a = run_bass_kernel_spmd(nc_a, in_a, core_ids).results
    tab = assemble_table(res_a, cfg)
    for m in in_b:
        m["tab"] = tab
    res_b = run_bass_kernel_spmd(nc_b, in_b, core_ids).results
    hcats = align_acc(res_b, perms, cfg)
    for c, m in enumerate(in_c):
        m["hcat"] = _bf16(hcats[c])
    res_c = run_bass_kernel_spmd(nc_c, in_c, core_ids).results
    return assemble_out(res_c, cfg)
